# revision 22
# baseline (speedup 1.0000x reference)
"""Multi-Head Latent Attention (MLA) prefill kernel for 8 Trainium2 NeuronCores.

Problem shapes: B=2, S=2048, DIM=2048, H=16 heads, HEAD_DIM=128, LATENT=512.

Strategy (tensor-parallel over heads + data-parallel phase A), v2:
  Phase A (token-DP): ropes + c_q (fp8 DR) computed FIRST so the small
    AllGather #2 (cq + ropes, fp8) launches ~10us in; c_kv (fp16) follows
    in two 256-token halves, each immediately AllGathered (AG1a/AG1b)
    carrying BOTH an fp16 payload (v path) and an fp8 copy (k up-proj) so
    phase B needs no per-rank converts. x8 converts run on Pool/DVE.
  Phase B (head-TP, 2 heads/core):
    kc/qc up-projections all-fp8 DoubleRow; v stays fp16 (accuracy) but is
    stored fp8 with a ones-column after PSUM evac.
    Scores: one DR fp8 matmul per 128-key chunk fusing content+rope
    (virtual K=256).
    exp: split across engines - most tiles on ACT (exp activation), a
    SCHR_TPS subset computed as Schraudolph bit-trick exp (DVE int32
    mul-add -> bitcast -> Pool fp8 convert), keeping the 137us/core ACT
    exp stream off the critical path.
    ctx: fp8 DR over adjacent key-chunk pairs (est8 x v8, virtual K=256),
    halving the fp16 ctx cost; softmax denominator rides the v8 ones
    column. Normalize + transpose (PE) + buv add as before.
    Out-projection fp16 (numerics); PSUM evacuations split DVE/ACT.
    Software pipeline: scores/exp of block i || ctx of i-1 || transposes
    of i-2 || deferred out-projections; up-proj remainders ride early
    block slack (late_work).
  Host: sums the 8 partial outputs and adds b_O.

fp8 scope validated by numpy e4m3 emulation (matches HW to <1e-3):
  baseline scheme 1.557e-2; + v8 single 1.754e-2; + Schraudolph on 3/8
  of exp tiles ~1.80e-2 vs the 2e-2 budget. ropes8/ckv8-compute rejected
  (>2e-2).
"""
import math

import numpy as np

import concourse.bacc as bacc
import concourse.mybir as mybir
import concourse.tile as tile
from concourse.masks import make_identity

# Problem constants (hardcoded per harness contract).
B, S, DIM, H, HD, LAT = 2, 2048, 2048, 16, 128, 512
N_CORES = 8
HPC = H // N_CORES          # heads per core = 2
TOK = B * S                 # 4096 tokens
TPC = TOK // N_CORES        # 512 tokens per core (phase A shard)
EC = DIM // 128             # 16 embedding chunks
LC = LAT // 128             # 4 latent chunks
RB = N_CORES                # rank blocks of TPC tokens
SBLK = 512                  # query block (phase B)
NSB = S // SBLK             # 4 s-blocks per batch
TC_B = S // 128             # 16 key chunks per batch
HALF = TPC // 2             # 256-token halves for the ckv AllGathers
F8 = mybir.dt.float8e4
F16 = mybir.dt.float16
F32 = mybir.dt.float32
I32 = mybir.dt.int32
SCALE = 1.0 / math.sqrt(HD)
DR = mybir.MatmulPerfMode.DoubleRow

# Schraudolph exp: exp(x) ~= bitcast_f32(int32(A*x + B')), C=0.0437
SCHR_A = float((1 << 23) / math.log(2.0)) * SCALE
SCHR_B = float(127.0 * (1 << 23) - 0.0437 * (1 << 23))
# which score-pair tiles (tp in 0..7) use Schraudolph instead of ACT exp
SCHR_TPS = (1, 5)
# out-projection evacuations: 5 of every 8 go to ACT, rest to DVE
OUT_EVAC_ACT = (True, False, True, False, True, False, True, False)

_CACHE = {}

OPTS = dict()


def _build(use_cc=True, n_devices=N_CORES, reps=1, tiny_out=False,
           **opts):
    nc = bacc.Bacc("TRN2", target_bir_lowering=False, debug=False,
                   num_devices=n_devices)

    # ---- per-core external inputs (host pre-permuted into SBUF layout so
    # every input DMA is a contiguous [128, X] copy) ----
    xT = nc.dram_tensor("xT", [128, EC, TPC], F16, kind="ExternalInput")
    wdkv = nc.dram_tensor("wdkv", [128, EC, LAT], F16, kind="ExternalInput")
    wdq = nc.dram_tensor("wdq", [128, EC, LAT], F8, kind="ExternalInput")
    wkr = nc.dram_tensor("wkr", [128, EC, HD], F16, kind="ExternalInput")
    wqr = nc.dram_tensor("wqr", [128, EC, HD], F16, kind="ExternalInput")
    bdkv = nc.dram_tensor("bdkv", [128, LC], F32, kind="ExternalInput")
    bdq = nc.dram_tensor("bdq", [128, LC], F32, kind="ExternalInput")
    bkr = nc.dram_tensor("bkr", [128, 1], F32, kind="ExternalInput")
    bqr = nc.dram_tensor("bqr", [128, 1], F32, kind="ExternalInput")
    pet = nc.dram_tensor("pet", [HD, TPC], F32, kind="ExternalInput")
    wuk = nc.dram_tensor("wuk", [128, LC, HPC * HD], F8,
                         kind="ExternalInput")
    wuv = nc.dram_tensor("wuv", [128, LC, HPC * HD], F16,
                         kind="ExternalInput")
    wuq = nc.dram_tensor("wuq", [128, LC, HPC * HD], F8,
                         kind="ExternalInput")
    buk = nc.dram_tensor("buk", [128, HPC], F32, kind="ExternalInput")
    buv = nc.dram_tensor("buv", [128, HPC], F32, kind="ExternalInput")
    buq = nc.dram_tensor("buq", [128, HPC], F32, kind="ExternalInput")
    wo = nc.dram_tensor("wo", [128, HPC, DIM], F16, kind="ExternalInput")

    # timing-only builds use a tiny aliased output to avoid the host
    # cost of 16MB/core output buffers per dispatch
    out_shape = [128, DIM] if tiny_out else [TOK, DIM]
    out_part = nc.dram_tensor("out_part", out_shape, F16,
                              kind="ExternalOutput")

    # bounce-buffer sections, in f16-element units
    CKV16_N = 128 * LC * HALF          # fp16 ckv half payload
    CKV8_S = CKV16_N // 2              # fp8 ckv half payload (f16 slots)
    B1_N = CKV16_N + CKV8_S
    CQ_S = 128 * LC * TPC // 2         # fp8 c_q (f16 slots)
    R_S = 128 * TPC // 2               # fp8 rope tensor (f16 slots)
    KR_OFF = CQ_S
    QR_OFF = CQ_S + R_S
    AUX_N = CQ_S + 2 * R_S

    with tile.TileContext(nc) as tc:
        def emit(rep):
            with tc.tile_pool(name=f"dram{rep}", bufs=1, space="DRAM") as dram:
                bin1a = dram.tile([1, B1_N], F16)
                bout1a = dram.tile([RB, B1_N], F16, addr_space="Shared")
                bin1b = dram.tile([1, B1_N], F16)
                bout1b = dram.tile([RB, B1_N], F16, addr_space="Shared")
                bin2 = dram.tile([1, AUX_N], F16)
                bout2 = dram.tile([RB, AUX_N], F16, addr_space="Shared")

                _pB0_cm = tc.tile_pool(name=f"pB0{rep}", bufs=1)
                pb0 = _pB0_cm.__enter__()
                # persistent phase-B tiles; weight DMAs are emitted inside
                # phase A (after the phase-A weights) so the scalar DGE
                # queue serves phase A first
                wuk_sb = pb0.tile([128, LC, HPC * HD], F8)
                wuv_sb = pb0.tile([128, LC, HPC * HD], F16)
                wuq_sb = pb0.tile([128, LC, HPC * HD], F8)
                wo_sb = pb0.tile([128, HPC, DIM], F16)
                buk_sb = pb0.tile([128, HPC], F32)
                buv_sb = pb0.tile([128, HPC], F32)
                buq_sb = pb0.tile([128, HPC], F32)
                ident = pb0.tile([128, 128], F16)
                kk_sb = pb0.tile([128, 3, RB, TPC], F8)
                qq_sb = pb0.tile([128, 3, RB, TPC], F8)
                v8_sb = pb0.tile([128, HPC, TOK // 128, 132], F8)
                ckv_sb = pb0.tile([128, LC, RB, TPC], F16)
                ckv8_sb = pb0.tile([128, LC, RB, TPC], F8)
                cq_sb = pb0.tile([128, LC, RB, TPC], F8)

                # ============ Phase A (token shard, transposed outputs) ====
                with tc.tile_pool(name=f"pA{rep}", bufs=1) as pA, \
                     tc.tile_pool(name=f"psA{rep}", bufs=1,
                                  space="PSUM") as psA:
                    # x streams on the SP queue; weights on the ACT queue
                    x_sb = pA.tile([128, EC, TPC], F16)
                    chunks = [(0, 1), (1, 1), (2, 2), (4, 2), (6, 2),
                              (8, 2), (10, 2), (12, 2), (14, 2)]
                    for _c, _w in chunks:
                        nc.sync.dma_start(out=x_sb[:, _c:_c + _w],
                                          in_=xT[:, _c:_c + _w, :])
                    wkr_sb = pA.tile([128, EC, HD], F16)
                    wqr_sb = pA.tile([128, EC, HD], F16)
                    for _c, _w in ((0, 2), (2, 6), (8, 8)):
                        nc.scalar.dma_start(out=wkr_sb[:, _c:_c + _w],
                                            in_=wkr[:, _c:_c + _w, :])
                        nc.scalar.dma_start(out=wqr_sb[:, _c:_c + _w],
                                            in_=wqr[:, _c:_c + _w, :])
                    wdq_sb = pA.tile([128, EC, LAT], F8)
                    for _c in range(0, EC, 4):
                        nc.scalar.dma_start(out=wdq_sb[:, _c:_c + 4],
                                            in_=wdq[:, _c:_c + 4, :])
                    wdkv_sb = pA.tile([128, EC, LAT], F16)
                    for _c in range(0, EC, 2):
                        nc.scalar.dma_start(out=wdkv_sb[:, _c:_c + 2],
                                            in_=wdkv[:, _c:_c + 2, :])
                    # light phase-B weights follow; wuv/wo are deferred
                    # behind the first bounce write (same queue)
                    nc.scalar.dma_start(out=wuk_sb, in_=wuk[:, :, :])
                    nc.scalar.dma_start(out=wuq_sb, in_=wuq[:, :, :])
                    nc.scalar.dma_start(out=wuv_sb, in_=wuv[:, :, :])
                    # small tensors on the SP queue
                    bdkv_sb = pA.tile([128, LC], F32)
                    nc.sync.dma_start(out=bdkv_sb, in_=bdkv[:, :])
                    bdq_sb = pA.tile([128, LC], F32)
                    nc.sync.dma_start(out=bdq_sb, in_=bdq[:, :])
                    bkr_sb = pA.tile([128, 1], F32)
                    nc.sync.dma_start(out=bkr_sb, in_=bkr[:, :])
                    bqr_sb = pA.tile([128, 1], F32)
                    nc.sync.dma_start(out=bqr_sb, in_=bqr[:, :])
                    pet_sb = pA.tile([128, TPC], F32)
                    nc.sync.dma_start(out=pet_sb, in_=pet[:, :])
                    nc.sync.dma_start(out=buk_sb, in_=buk[:, :])
                    nc.sync.dma_start(out=buv_sb, in_=buv[:, :])
                    nc.sync.dma_start(out=buq_sb, in_=buq[:, :])

                    make_identity(nc, ident)
                    nc.gpsimd.memset(v8_sb[:, :, :, 128:129], 1.0)

                    # x8 converts (Pool/DVE, both idle here) per 2-chunk
                    x8_sb = pA.tile([128, EC, TPC], F8)
                    for g in range(EC // 2):
                        eng = nc.gpsimd if g % 2 == 0 else nc.vector
                        eng.tensor_copy(x8_sb[:, 2 * g:2 * g + 2],
                                        x_sb[:, 2 * g:2 * g + 2])

                    # ---- ropes (fp16) + c_q (fp8 DR), ec-pair outer ----
                    ps_kr = psA.tile([128, TPC], F32, tag="psa_kr")
                    ps_qr = psA.tile([128, TPC], F32, tag="psa_qr")
                    ps_q = [psA.tile([128, TPC], F32, tag=f"psa{lc}",
                                     name=f"ps_q{lc}")
                            for lc in range(LC)]
                    for g in range(EC // 2):
                        for e in (2 * g, 2 * g + 1):
                            nc.tensor.matmul(ps_kr[:, :], wkr_sb[:, e, :],
                                             x_sb[:, e, :],
                                             start=(e == 0),
                                             stop=(e == EC - 1))
                            nc.tensor.matmul(ps_qr[:, :], wqr_sb[:, e, :],
                                             x_sb[:, e, :],
                                             start=(e == 0),
                                             stop=(e == EC - 1))
                        ep = slice(2 * g, 2 * g + 2)
                        for lc in range(LC):
                            nc.tensor.matmul(
                                ps_q[lc][:, :],
                                wdq_sb[:, ep, lc * 128:(lc + 1) * 128],
                                x8_sb[:, ep, :],
                                start=(g == 0), stop=(g == EC // 2 - 1),
                                perf_mode=DR)

                    krT8_sb = pA.tile([128, TPC], F8)
                    qrT8_sb = pA.tile([128, TPC], F8)
                    tmpr = pA.tile([128, TPC], F32)
                    nc.scalar.add(tmpr[:, :], ps_kr[:, :], bkr_sb[:, 0:1])
                    nc.vector.tensor_mul(krT8_sb[:, :], tmpr[:, :],
                                         pet_sb[:, :])
                    tmpr2 = pA.tile([128, TPC], F32)
                    nc.scalar.add(tmpr2[:, :], ps_qr[:, :], bqr_sb[:, 0:1])
                    nc.vector.tensor_mul(qrT8_sb[:, :], tmpr2[:, :],
                                         pet_sb[:, :])
                    cqT8_sb = pA.tile([128, LC, TPC], F8)
                    for lc in range(LC):
                        nc.scalar.add(cqT8_sb[:, lc, :], ps_q[lc][:, :],
                                      bdq_sb[:, lc:lc + 1])

                    nc.sync.dma_start(
                        out=bin2[0, 0:CQ_S].bitcast(F8).rearrange(
                            "(p n f) -> p n f", p=128, f=TPC),
                        in_=cqT8_sb)
                    nc.sync.dma_start(
                        out=bin2[0, KR_OFF:QR_OFF].bitcast(F8).rearrange(
                            "(p f) -> p f", p=128), in_=krT8_sb)
                    nc.sync.dma_start(
                        out=bin2[0, QR_OFF:AUX_N].bitcast(F8).rearrange(
                            "(p f) -> p f", p=128), in_=qrT8_sb)
                    if use_cc:
                        nc.gpsimd.collective_compute(
                            "AllGather", mybir.AluOpType.bypass,
                            replica_groups=[list(range(N_CORES))],
                            ins=[bin2.opt()], outs=[bout2.opt()])

                    # ---- c_kv (fp16+fp8) in two token halves, AG each ----
                    ckvT_sb = pA.tile([128, LC, TPC], F16)
                    ckvT8_sb = pA.tile([128, LC, TPC], F8)

                    def ckv_half(half, bin1, bout1):
                        hs = slice(half * HALF, (half + 1) * HALF)
                        for ec in range(EC):
                            for lc in range(LC):
                                nc.tensor.matmul(
                                    ps_q[lc][:, hs],
                                    wdkv_sb[:, ec, lc * 128:(lc + 1) * 128],
                                    x_sb[:, ec, hs],
                                    start=(ec == 0), stop=(ec == EC - 1))
                        for lc in range(LC):
                            nc.scalar.add(ckvT_sb[:, lc, hs],
                                          ps_q[lc][:, hs],
                                          bdkv_sb[:, lc:lc + 1])
                        nc.gpsimd.tensor_copy(ckvT8_sb[:, :, hs],
                                              ckvT_sb[:, :, hs])
                        nc.scalar.dma_start(
                            out=bin1[0, 0:CKV16_N].rearrange(
                                "(p n f) -> p n f", p=128, f=HALF),
                            in_=ckvT_sb[:, :, hs])
                        nc.scalar.dma_start(
                            out=bin1[0, CKV16_N:B1_N].bitcast(F8).rearrange(
                                "(p n f) -> p n f", p=128, f=HALF),
                            in_=ckvT8_sb[:, :, hs])
                        if use_cc:
                            nc.gpsimd.collective_compute(
                                "AllGather", mybir.AluOpType.bypass,
                                replica_groups=[list(range(N_CORES))],
                                ins=[bin1.opt()], outs=[bout1.opt()])

                    def ckv_load(eng, half, bt, r):
                        hs = slice(half * HALF, (half + 1) * HALF)
                        eng.dma_start(
                            out=ckv8_sb[:, :, r, hs],
                            in_=bt[r, CKV16_N:B1_N].bitcast(F8).rearrange(
                                "(p n f) -> p n f", p=128, f=HALF))
                        eng.dma_start(
                            out=ckv_sb[:, :, r, hs],
                            in_=bt[r, 0:CKV16_N].rearrange(
                                "(p n f) -> p n f", p=128, f=HALF))

                    ckv_half(0, bin1a, bout1a)
                    # wo rides the scalar queue here (needed much later);
                    # then the warmup-critical halfA rank loads
                    nc.scalar.dma_start(out=wo_sb, in_=wo[:, :, :])
                    for r in range(4):
                        ckv_load(nc.scalar, 0, bout1a, r)
                    ckv_half(1, bin1b, bout1b)

                # ---- AG-dependent SBUF loads, in expected arrival order.
                # SP queue: AG2 payloads then the ckv halfB loads; ACT
                # queue (idle after weights + bin1 writes): halfA loads.
                nc.sync.dma_start(
                    out=kk_sb[:, 2, :, :],
                    in_=bout2[:, KR_OFF:QR_OFF].bitcast(F8).rearrange(
                        "r (p f) -> p r f", p=128))
                nc.sync.dma_start(
                    out=qq_sb[:, 2, :, :],
                    in_=bout2[:, QR_OFF:AUX_N].bitcast(F8).rearrange(
                        "r (p f) -> p r f", p=128))
                def cq_load(r):
                    nc.sync.dma_start(
                        out=cq_sb[:, :, r, :],
                        in_=bout2[r, 0:CQ_S].bitcast(F8).rearrange(
                            "(p n f) -> p n f", p=128, f=TPC))

                for r in range(2):
                    cq_load(r)
                for r in range(4):
                    ckv_load(nc.sync, 1, bout1b, r)
                for r in range(2, RB):
                    cq_load(r)
                for r in range(4, RB):
                    ckv_load(nc.scalar, 0, bout1a, r)
                for r in range(4, RB):
                    ckv_load(nc.sync, 1, bout1b, r)

                # ============ Phase B ======================================
                with tc.tile_pool(name=f"pAt{rep}", bufs=2) as pAt, \
                     tc.tile_pool(name=f"psU{rep}", bufs=1,
                                  space="PSUM") as psU, \
                     tc.tile_pool(name=f"psS{rep}", bufs=2,
                                  space="PSUM") as psS, \
                     tc.tile_pool(name=f"psC{rep}", bufs=1,
                                  space="PSUM") as psC:

                    def kc_rank(rb, half):
                        hs = slice(half * HALF, (half + 1) * HALF)
                        for h in range(HPC):
                            hsl = slice(h * HD, (h + 1) * HD)
                            psk = psU.tile([128, HALF], F32, tag="ps_u",
                                           name="psk", bufs=2)
                            for p2 in range(LC // 2):
                                lcs = slice(2 * p2, 2 * p2 + 2)
                                nc.tensor.matmul(
                                    psk[:, :], wuk_sb[:, lcs, hsl],
                                    ckv8_sb[:, lcs, rb, hs],
                                    start=(p2 == 0),
                                    stop=(p2 == LC // 2 - 1),
                                    perf_mode=DR)
                            nc.vector.tensor_scalar_add(
                                kk_sb[:, h, rb, hs], psk[:, :],
                                buk_sb[:, h:h + 1])

                    def qc_rank(rq):
                        for h in range(HPC):
                            hsl = slice(h * HD, (h + 1) * HD)
                            psq = psU.tile([128, TPC], F32, tag="ps_u",
                                           name="psq", bufs=2)
                            for p2 in range(LC // 2):
                                lcs = slice(2 * p2, 2 * p2 + 2)
                                nc.tensor.matmul(
                                    psq[:, :], wuq_sb[:, lcs, hsl],
                                    cq_sb[:, lcs, rq, :],
                                    start=(p2 == 0),
                                    stop=(p2 == LC // 2 - 1),
                                    perf_mode=DR)
                            nc.vector.tensor_scalar_add(
                                qq_sb[:, h, rq, :], psq[:, :],
                                buq_sb[:, h:h + 1])

                    def v_rank(rb, t4s):
                        for t4 in t4s:
                            psv = psU.tile([128, HPC * HD], F32,
                                           tag="ps_v", name="psv", bufs=1)
                            for lc in range(LC):
                                nc.tensor.matmul(
                                    psv[:, :],
                                    ckv_sb[:, lc, rb,
                                           t4 * 128:(t4 + 1) * 128],
                                    wuv_sb[:, lc, :],
                                    start=(lc == 0), stop=(lc == LC - 1))
                            nc.vector.tensor_copy(
                                v8_sb[:, :, rb * 4 + t4, 0:128],
                                psv[:, :])

                    out_dram = out_part.ap().rearrange(
                        "(n p) f -> p n f", p=128)
                    n_slots = 1 if tiny_out else TOK // 128

                    def pair(h):
                        # kk/qq slots {h, 2}: (kc_h, kr) / (qc_h, qr)
                        return slice(h, 3, 2 - h)

                    def issue_scores(b, sb4, h, est_sb=None,
                                     tps=(0, 1, 2, 3, 4, 5, 6, 7)):
                        """DR-fp8 scores + exp (ACT / Schraudolph split)."""
                        rq = 4 * b + sb4
                        if est_sb is None:
                            est_sb = pAt.tile([128, TC_B, SBLK], F8,
                                              tag="est", bufs=3)
                        for tp in tps:
                            ps_s = psS.tile([128, 2, SBLK], F32,
                                            tag="ps_s")
                            for ti in range(2):
                                t = 2 * tp + ti
                                rk = 4 * b + t // 4
                                ko = (t % 4) * 128
                                nc.tensor.matmul(
                                    ps_s[:, ti, :],
                                    kk_sb[:, pair(h), rk, ko:ko + 128],
                                    qq_sb[:, pair(h), rq, :],
                                    start=True, stop=True, perf_mode=DR)
                            eslot = est_sb[:, 2 * tp:2 * (tp + 1), :]
                            if tp in SCHR_TPS:
                                e32 = pAt.tile([128, 2, SBLK], I32,
                                               tag="est32", bufs=2)
                                nc.vector.tensor_scalar(
                                    e32, ps_s[:, :, :], SCHR_A, SCHR_B,
                                    mybir.AluOpType.mult,
                                    mybir.AluOpType.add)
                                nc.gpsimd.tensor_copy(
                                    eslot, e32[:, :, :].bitcast(F32))
                            else:
                                nc.scalar.activation(
                                    eslot, ps_s[:, :, :],
                                    mybir.ActivationFunctionType.Exp,
                                    scale=SCALE)
                        return est_sb

                    def issue_ctx_chains(b, sb4, h, est_sb):
                        """fp8 DR ctx chains (key-chunk pairs) + normalize."""
                        muls = []
                        for sp in range(SBLK // 256):
                            ps_c = psC.tile([128, 2, 132], F32,
                                            tag="ps_c")
                            for si in range(2):
                                ss = 2 * sp + si
                                for tp in range(TC_B // 2):
                                    nc.tensor.matmul(
                                        ps_c[:, si, 0:129],
                                        est_sb[:, 2 * tp:2 * tp + 2,
                                               ss * 128:(ss + 1) * 128],
                                        v8_sb[:, h,
                                              TC_B * b + 2 * tp:
                                              TC_B * b + 2 * tp + 2,
                                              0:129],
                                        start=(tp == 0),
                                        stop=(tp == TC_B // 2 - 1),
                                        perf_mode=DR)
                                recip = pAt.tile([128, 1], F32,
                                                 tag="recip", bufs=8)
                                nc.vector.reciprocal(
                                    recip, ps_c[:, si, 128:129])
                                ctxn_sb = pAt.tile([128, 128], F16,
                                                   tag="ctxn", bufs=8)
                                nc.vector.tensor_scalar_mul(
                                    ctxn_sb[:, :], ps_c[:, si, 0:128],
                                    recip)
                                muls.append((ss, ctxn_sb))
                        return muls

                    def issue_transposes(h, ctxT_sb, muls):
                        # deferred a full block: the DVE muls are long
                        # done, so the PE never waits here. Scratch
                        # borrows the ps_v bank (v-path is idle then).
                        sc = psU.tile([128, HPC * HD], F32, tag="ps_v",
                                      name="sc", bufs=1)
                        for i, (ss, ctxn_sb) in enumerate(muls):
                            scratch = sc[:, (i % 2) * 64:
                                         (i % 2) * 64 + 64].bitcast(F16)
                            nc.tensor.transpose(scratch, ctxn_sb[:, :],
                                                ident[:, :])
                            nc.vector.tensor_scalar_add(
                                ctxT_sb[:, h, ss, :], scratch,
                                buv_sb[:, h:h + 1])

                    def issue_outproj(b, sb4, ctxT_sb, fine=False):
                        rq = 4 * b + sb4
                        n0 = (rq * TPC) // 128
                        for ss in range(SBLK // 128):
                            out_sb = pAt.tile([128, DIM], F16,
                                              tag="out", bufs=3)
                            for dt4 in range(DIM // 512):
                                ps_o = psU.tile([128, TPC], F32,
                                                tag="ps_u", name="ps_o",
                                                bufs=2)
                                for h in range(HPC):
                                    nc.tensor.matmul(
                                        ps_o[:, :],
                                        ctxT_sb[:, h, ss, :],
                                        wo_sb[:, h,
                                              dt4 * 512:(dt4 + 1) * 512],
                                        start=(h == 0),
                                        stop=(h == HPC - 1))
                                osl = out_sb[:, dt4 * 512:(dt4 + 1) * 512]
                                if OUT_EVAC_ACT[(ss * 4 + dt4) % 8]:
                                    nc.scalar.copy(osl, ps_o[:, :])
                                else:
                                    nc.vector.tensor_copy(osl, ps_o[:, :])
                                if fine:
                                    qeng = nc.sync if dt4 % 2 else nc.scalar
                                    qeng.dma_start(
                                        out=out_dram[
                                            :, (n0 + ss) % n_slots,
                                            dt4 * 512:(dt4 + 1) * 512],
                                        in_=osl)
                            if not fine:
                                nc.sync.dma_start(
                                    out=out_dram[:, (n0 + ss) % n_slots, :],
                                    in_=out_sb)

                    # ---- warmup, ordered by expected data arrival:
                    # AG2+AG1a -> halfA up-proj + even-tp scores of the
                    # first block pair; AG1b -> halfB + odd-tp scores.
                    for rb in range(4):
                        kc_rank(rb, 0)
                        v_rank(rb, (0, 1))
                    qc_rank(0)
                    EV, OD = (0, 2, 4, 6), (1, 3, 5, 7)
                    ctxT0 = pAt.tile([128, HPC, SBLK // 128, 128], F16,
                                     tag="ctxT", bufs=4)
                    est00 = issue_scores(0, 0, 0, tps=EV)
                    est01 = issue_scores(0, 0, 1, tps=EV)
                    for rb in range(4):
                        kc_rank(rb, 1)
                        v_rank(rb, (2, 3))
                    qc_rank(1)
                    issue_scores(0, 0, 0, est_sb=est00, tps=OD)
                    issue_scores(0, 0, 1, est_sb=est01, tps=OD)
                    muls0 = issue_ctx_chains(0, 0, 0, est00)

                    late_work = {
                        2: [lambda: qc_rank(2)],
                        3: [lambda: qc_rank(3)],
                        4: [lambda: qc_rank(4), lambda: kc_rank(4, 0),
                            lambda: kc_rank(4, 1), lambda: v_rank(4, (0, 1)),
                            lambda: v_rank(4, (2, 3))],
                        5: [lambda: kc_rank(5, 0),
                            lambda: kc_rank(5, 1), lambda: v_rank(5, (0, 1)),
                            lambda: v_rank(5, (2, 3))],
                        6: [lambda: kc_rank(6, 0),
                            lambda: kc_rank(6, 1), lambda: v_rank(6, (0, 1)),
                            lambda: v_rank(6, (2, 3))],
                        7: [lambda: kc_rank(7, 0),
                            lambda: kc_rank(7, 1), lambda: v_rank(7, (0, 1)),
                            lambda: v_rank(7, (2, 3))],
                        8: [lambda: qc_rank(5)],
                        10: [lambda: qc_rank(6)],
                        12: [lambda: qc_rank(7)],
                    }

                    blocks = [(b, sb4, h) for b in range(B)
                              for sb4 in range(NSB) for h in range(HPC)]
                    # pipeline state primed by the warmup
                    prev = (0, 0, 1, est01, ctxT0)   # awaiting ctx chains
                    prev2 = (0, ctxT0, muls0, 0, 0)  # awaiting transposes
                    ctxT_sb = ctxT0
                    pending_out = []
                    for i, blk in enumerate(blocks):
                        if i < 2:
                            continue
                        b, sb4, h = blk
                        if h == 0:
                            ctxT_sb = pAt.tile(
                                [128, HPC, SBLK // 128, 128], F16,
                                tag="ctxT", bufs=4)
                        est = issue_scores(b, sb4, h)
                        for piece in late_work.get(i, []):
                            piece()
                        if prev is not None:
                            pb, psb4, ph, pest, pctxT = prev
                            pmuls = issue_ctx_chains(pb, psb4, ph, pest)
                            if prev2 is not None:
                                p2h, p2ctxT, p2muls, p2b, p2sb4 = prev2
                                issue_transposes(p2h, p2ctxT, p2muls)
                                if p2h == 1:
                                    pending_out.append(
                                        (p2b, p2sb4, p2ctxT))
                            prev2 = (ph, pctxT, pmuls, pb, psb4)
                        if i >= 6 and pending_out:
                            issue_outproj(*pending_out.pop(0))
                        prev = (b, sb4, h, est, ctxT_sb)
                    pb, psb4, ph, pest, pctxT = prev
                    pmuls = issue_ctx_chains(pb, psb4, ph, pest)
                    p2h, p2ctxT, p2muls, p2b, p2sb4 = prev2
                    issue_transposes(p2h, p2ctxT, p2muls)
                    if p2h == 1:
                        pending_out.append((p2b, p2sb4, p2ctxT))
                    issue_transposes(ph, pctxT, pmuls)
                    pending_out.append((pb, psb4, pctxT))
                    while pending_out:
                        issue_outproj(*pending_out.pop(0),
                                      fine=(len(pending_out) == 0))
                _pB0_cm.__exit__(None, None, None)

        for rep in range(reps):
            emit(rep)
    nc.compile()
    return nc


def _rope_pe():
    pos = np.arange(S, dtype=np.float32)[:, None]
    div = np.exp(np.arange(0, HD, 2, dtype=np.float32)
                 * (-math.log(10000.0) / HD))
    pe = np.zeros((S, HD), dtype=np.float32)
    pe[:, 0::2] = np.sin(pos * div)
    pe[:, 1::2] = np.cos(pos * div)
    return pe


def _sbl(w, f16=True):
    """[n*128, C...] -> SBUF layout [128, n, C...] (partition-major)."""
    w = np.asarray(w, np.float32)
    n = w.shape[0] // 128
    out = np.ascontiguousarray(
        w.reshape(n, 128, *w.shape[1:]).swapaxes(0, 1))
    return out.astype(np.float16) if f16 else out


def _sblb(b):
    """bias [n*128] -> [128, n] fp32."""
    b = np.asarray(b, np.float32)
    n = b.size // 128
    return np.ascontiguousarray(b.reshape(n, 128).T)


def _to8(a):
    """fp32 -> TRN e4m3 (ml_dtypes.float8_e4m3, clipped to +-240)."""
    import ml_dtypes
    return np.clip(np.asarray(a, np.float32), -240.0, 240.0).astype(
        ml_dtypes.float8_e4m3)


def _prep_in_maps(inputs):
    f16 = np.float16
    x = np.asarray(inputs["x"], np.float32).reshape(TOK, DIM)
    pe = _rope_pe()
    wdq_l = _sbl(inputs["W_DQ"], f16=False)
    shared = dict(
        wdkv=_sbl(inputs["W_DKV"]),
        wdq=_to8(wdq_l),
        wkr=_sbl(inputs["W_KR"]),
        wqr=_sbl(inputs["W_QR"]),
        bdkv=_sblb(inputs["b_DKV"]),
        bdq=_sblb(inputs["b_DQ"]),
        bkr=_sblb(inputs["b_KR"]),
        bqr=_sblb(inputs["b_QR"]),
    )
    in_maps = []
    for r in range(N_CORES):
        tok = slice(r * TPC, (r + 1) * TPC)
        hslice = slice(r * HPC * HD, (r + 1) * HPC * HD)
        pos0 = (r * TPC) % S
        m = dict(shared)
        # xT sbuf layout: [128, EC, TPC]; x_sb[p, n, f] = x[tok_f, n*128+p]
        m["xT"] = np.ascontiguousarray(
            x[tok].T.reshape(EC, 128, TPC).swapaxes(0, 1)).astype(f16)
        m["pet"] = np.ascontiguousarray(pe[pos0:pos0 + TPC].T)
        wuk_l = _sbl(np.asarray(inputs["W_UK"], np.float32)[:, hslice],
                     f16=False)
        wuq_l = _sbl(np.asarray(inputs["W_UQ"], np.float32)[:, hslice],
                     f16=False)
        m["wuk"] = _to8(wuk_l)
        m["wuq"] = _to8(wuq_l)
        m["wuv"] = _sbl(np.asarray(inputs["W_UV"], np.float32)[:, hslice])
        m["buk"] = _sblb(np.asarray(inputs["b_UK"], np.float32)[hslice])
        m["buv"] = _sblb(np.asarray(inputs["b_UV"], np.float32)[hslice])
        m["buq"] = _sblb(np.asarray(inputs["b_UQ"], np.float32)[hslice])
        m["wo"] = _sbl(np.asarray(inputs["W_O"], np.float32)[hslice, :])
        in_maps.append(m)
    return in_maps


def _build_single(**opts):
    """Single-core, collective-free variant for cost-model timing."""
    return _build(use_cc=False, n_devices=1, **opts)


def _get_exec():
    """Build (once) a jitted shard_map executor over the 8 cores, mirroring
    concourse.bass2jax.run_bass_via_pjrt but cached so repeated kernel()
    calls do not re-trace/re-compile."""
    if "exec" in _CACHE:
        return _CACHE["exec"]
    import jax
    from jax.sharding import Mesh, PartitionSpec, NamedSharding
    from jax.experimental.shard_map import shard_map
    from concourse import bass2jax

    bass2jax.install_neuronx_cc_hook()
    if "nc" not in _CACHE:
        _CACHE["nc"] = _build()
    nc = _CACHE["nc"]

    _pname = nc.partition_id_tensor.name if nc.partition_id_tensor else None
    in_names, out_names, out_avals, zero_outs = [], [], [], []
    for alloc in nc.m.functions[0].allocations:
        if not isinstance(alloc, mybir.MemoryLocationSet):
            continue
        name = alloc.memorylocations[0].name
        if alloc.kind == "ExternalInput":
            if name != _pname:
                in_names.append(name)
        elif alloc.kind == "ExternalOutput":
            out_names.append(name)
            shape = tuple(alloc.tensor_shape)
            dtype = mybir.dt.np(alloc.dtype)
            out_avals.append(jax.core.ShapedArray(shape, dtype))
            zero_outs.append(np.zeros((N_CORES * shape[0], *shape[1:]), dtype))
    n_params = len(in_names)
    partition_name = (nc.partition_id_tensor.name
                      if nc.partition_id_tensor else None)
    all_names = in_names + out_names
    if partition_name is not None:
        all_names = all_names + [partition_name]

    def _body(*args):
        operands = list(args)
        if partition_name is not None:
            operands.append(bass2jax.partition_id_tensor())
        outs = bass2jax._bass_exec_p.bind(
            *operands,
            out_avals=tuple(out_avals),
            in_names=tuple(all_names),
            out_names=tuple(out_names),
            lowering_input_output_aliases=(),
            sim_require_finite=True,
            sim_require_nnan=True,
            nc=nc,
        )
        return tuple(outs)

    devices = jax.devices()[:N_CORES]
    mesh = Mesh(np.asarray(devices), ("core",))
    spec = PartitionSpec("core")
    in_specs = (spec,) * (n_params + len(out_names))
    out_specs = (spec,) * len(out_names)
    sharded = jax.jit(
        shard_map(_body, mesh=mesh, in_specs=in_specs, out_specs=out_specs,
                  check_rep=False),
        keep_unused=True,
    )
    sharding = NamedSharding(mesh, spec)
    zeros_dev = [jax.device_put(z, sharding) for z in zero_outs]
    _CACHE["exec"] = (sharded, in_names, out_names, out_avals, zeros_dev,
                      sharding)
    return _CACHE["exec"]


def _execute(in_maps):
    import jax
    sharded, in_names, out_names, out_avals, zeros_dev, sharding = _get_exec()
    concat_in = [
        np.concatenate([np.asarray(in_maps[c][n]) for c in range(N_CORES)],
                       axis=0)
        for n in in_names
    ]
    dev_in = [jax.device_put(a, sharding) for a in concat_in]
    out_arrs = sharded(*dev_in, *zeros_dev)
    out_arrs = [np.asarray(o) for o in out_arrs]
    return [
        {n: out_arrs[i].reshape(N_CORES, *out_avals[i].shape)[c]
         for i, n in enumerate(out_names)}
        for c in range(N_CORES)
    ]


def run(**inputs):
    in_maps = _prep_in_maps(inputs)
    results = _execute(in_maps)
    acc = np.zeros((TOK, DIM), np.float32)
    for r in range(N_CORES):
        acc += results[r]["out_part"].astype(np.float32)
    acc += np.asarray(inputs["b_O"], np.float32)
    return acc.reshape(B, S, DIM), results


def exec_only(in_maps):
    """For timing: run the prebuilt executor on preprocessed inputs."""
    return _execute(in_maps)


def timeit(inputs, n=10):
    """Time the device execution with device-resident inputs (excludes
    host prep and H2D transfer; includes PJRT/tunnel dispatch)."""
    import time
    import jax
    in_maps = _prep_in_maps(inputs)
    sharded, in_names, _, _, zeros_dev, sharding = _get_exec()
    dev_in = [
        jax.device_put(
            np.concatenate([np.asarray(in_maps[c][nm])
                            for c in range(N_CORES)], axis=0), sharding)
        for nm in in_names
    ]
    outs = sharded(*dev_in, *zeros_dev)   # warm-up
    jax.block_until_ready(outs)
    times = []
    for _ in range(n):
        t0 = time.perf_counter()
        outs = sharded(*dev_in, *zeros_dev)
        jax.block_until_ready(outs)
        times.append(time.perf_counter() - t0)
    return times


def kernel(**inputs):
    out, _ = run(**inputs)
    return out
